# revision 22
# baseline (speedup 1.0000x reference)
"""BGCF layer forward on 8 Trainium2 NeuronCores (Bass/Tile).

Key observation: the reference only reads item-side aggregates at
pos_items/neg_items (and obs_* for the observed graph), and user-side
quantities at users/obs_users. So instead of streaming the full
8192x8192 adjacency matrices, each core processes
  - the 2048 gathered item COLUMNS (pos|neg) of its 1024-user row shard
    -> partial (A_cols.T @ [ue|1]) reduced across cores with ONE fused
       ReduceScatter (column degree rides along as the ones column),
  - the 128 gathered batch user ROWS for the attention (score) pass.

Adjacency data moves as fp8_e4m3 (0/1 values are exact), embeddings as
bf16; PSUM accumulation is fp32. The degree division commutes past the
64x64 weight matmuls: the column pass right-multiplies [ue @ W | 1]
(ue@W2 / ue@W_obs computed on device first), so the ReduceScatter
output is final up to an elementwise degree divide done on the host
alongside the reference's tanh/l2norm postprocessing. Schedule:
column-pass inputs load first so the ReduceScatter issues as early as
possible; the score pass (software-pipelined inner matmul -> exp ->
mask -> accumulate) streams in behind them and fills the collective's
window; the only post-collective device work is one DRAM copy of the
67KB reduced shard to the output tensor.
"""
import numpy as np
import ml_dtypes

import concourse.bacc as bacc
import concourse.tile as tile
import concourse.mybir as mybir
from concourse.bass_utils import run_bass_kernel_spmd

F32 = mybir.dt.float32
BF16 = mybir.dt.bfloat16
FP8 = mybir.dt.float8e4
ACT = mybir.ActivationFunctionType

NPBF16 = ml_dtypes.bfloat16
NPFP8 = ml_dtypes.float8_e4m3

M = 8            # cores
U = 8192         # users
I = 8192         # items
D = 64
B = 1024         # batch
USH = U // M     # user rows per core
BSH = B // M     # batch rows per core
NI = 2048        # gathered item columns per adjacency (pos|neg)
NG = 16          # score-pass item groups of 512
EPS = 1e-6

_CACHE = {}


def _build():
    nc = bacc.Bacc("TRN2", target_bir_lowering=False, debug=False, num_devices=M)

    ACOL = nc.dram_tensor("ACOL", [USH, NI], FP8, kind="ExternalInput")
    OCOL = nc.dram_tensor("OCOL", [USH, NI], FP8, kind="ExternalInput")
    # ue[core shard].T packed two 512-user halves tall
    UET2 = nc.dram_tensor("UET2", [128, USH // 2], BF16, kind="ExternalInput")
    # ie.T packed two item-halves tall so the DMA uses all 128 partitions:
    # partitions 0:64 = items 0:4096, partitions 64:128 = items 4096:8192
    IETP = nc.dram_tensor("IETP", [128, I // 2], BF16, kind="ExternalInput")
    # packed constants: [ue[users].T x2 | identity | W1 W2 WOBS WSEL]
    PK = nc.dram_tensor("PK", [128, 512], BF16, kind="ExternalInput")
    AUTP = nc.dram_tensor("AUTP", [I // 4, 512], FP8, kind="ExternalInput")
    OUTP = nc.dram_tensor("OUTP", [I // 4, 512], FP8, kind="ExternalInput")
    IEAP = nc.dram_tensor("IEAP", [I // 4, 4 * (D + 1)], BF16, kind="ExternalInput")

    H1G = nc.dram_tensor("H1G", [BSH, D], F32, kind="ExternalOutput")
    H2G = nc.dram_tensor("H2G", [BSH, D], F32, kind="ExternalOutput")
    OBSG = nc.dram_tensor("OBSG", [BSH, D], F32, kind="ExternalOutput")

    # pack4 layout: row g*128+p holds items {512g+128q+p : q} x 65 feats,
    # so store/readback descriptors are 520B and the flat ReduceScatter
    # scatter boundary lands exactly on each core's 512-item shard
    AGG = nc.dram_tensor("AGG", [2 * NI // 4, 4 * (D + 1)], BF16)
    RSO = nc.dram_tensor("RSO", [2 * NI // M // 4, 4 * (D + 1)], BF16)
    RSOUT = nc.dram_tensor("RSOUT", [2 * NI // M // 4, 4 * (D + 1)], BF16,
                           kind="ExternalOutput")

    with tile.TileContext(nc) as tc:
        with tc.tile_pool(name="per", bufs=1) as per, \
             tc.tile_pool(name="strip", bufs=2) as stripp, \
             tc.tile_pool(name="st", bufs=3) as stp, \
             tc.tile_pool(name="fin", bufs=2) as finp, \
             tc.tile_pool(name="pcol", bufs=2, space="PSUM") as pcol, \
             tc.tile_pool(name="pinner", bufs=5, space="PSUM") as pinner, \
             tc.tile_pool(name="pacc", bufs=1, space="PSUM") as pacc:

            # ---- column-pass inputs first: they gate the ReduceScatter ----
            # W2/WOBS fold into the pass: rhs = [ue @ W | 1], so the
            # ReduceScatter output only needs a host-side degree divide
            pk_sb = per.tile([128, 512], BF16, tag="pk")
            nc.sync.dma_start(pk_sb[:], PK[:, :])
            uegt_sb = pk_sb[:, 0:128]
            id_sb = pk_sb[:, 128:256]
            w1_sb, w2_sb, wobs_sb = (
                pk_sb[0:D, 256 + k * D:256 + (k + 1) * D] for k in range(3))
            uet_sb = per.tile([128, USH // 2], BF16, tag="uet")
            nc.sync.dma_start(uet_sb[:], UET2[:, :])
            NU = USH // 128
            uea2_sb = per.tile([128, 2, 4, D + 1], BF16, tag="uea2")
            uea3_sb = per.tile([128, 2, 4, D + 1], BF16, tag="uea3")
            for dst, wof in ((uea2_sb, 256 + D), (uea3_sb, 256 + 2 * D)):
                nc.vector.memset(dst[:, :, :, D:D + 1], 1.0)
                for u in range(NU):
                    pb = 0 if u < 4 else 64
                    pw = pcol.tile([128, D], F32, tag="pc")
                    nc.tensor.matmul(
                        pw[:], uet_sb[pb:pb + 64, (u % 4) * 128:(u % 4 + 1) * 128],
                        pk_sb[pb:pb + 64, wof:wof + D], start=True, stop=True)
                    nc.vector.tensor_copy(dst[:, u // 4, u % 4, 0:D], pw[:])
            acol, ocol = [], []
            for c in range(4):
                t = per.tile([128, NU, 512], FP8, tag=f"acol{c}")
                nc.sync.dma_start(
                    t[:], ACOL[:, c * 512:(c + 1) * 512].rearrange(
                        "(u p) i -> p u i", p=128))
                acol.append(t)
            for c in range(4):
                t = per.tile([128, NU, 512], FP8, tag=f"ocol{c}")
                nc.sync.dma_start(
                    t[:], OCOL[:, c * 512:(c + 1) * 512].rearrange(
                        "(u p) i -> p u i", p=128))
                ocol.append(t)

            # ---- score-pass loads, streamed behind the column inputs ----
            iet_sb = per.tile([128, I // 2], BF16, tag="iet")
            nc.sync.dma_start(iet_sb[:], IETP[:, :])
            aut_g, iea_g, out_g = [], [], []
            for gg in range(4):
                t = per.tile([128, 4, 512], FP8, tag=f"aut{gg}")
                nc.sync.dma_start(
                    t[:], AUTP[gg * 512:(gg + 1) * 512, :].rearrange(
                        "(g p) x -> p g x", p=128))
                aut_g.append(t)
                t = per.tile([128, 4, 4 * (D + 1)], BF16, tag=f"iea{gg}")
                nc.sync.dma_start(
                    t[:], IEAP[gg * 512:(gg + 1) * 512, :].rearrange(
                        "(g p) x -> p g x", p=128))
                iea_g.append(t)
                t = per.tile([128, 4, 512], FP8, tag=f"out{gg}")
                nc.sync.dma_start(
                    t[:], OUTP[gg * 512:(gg + 1) * 512, :].rearrange(
                        "(g p) x -> p g x", p=128))
                out_g.append(t)

            # ---- column pass: AGG[base+i] = cols.T @ [ue|1] ----
            # strip stores ride the Act HWDGE queue; the Pool queue carries
            # only the ReduceScatter so it issues the moment stores land
            def col_pass(src, csz, uea_sb, rbase):
                for c in range(4):
                    strip = stripp.tile([128, 4, D + 1], BF16, tag="strip")
                    for tt in range(4):
                        col = c * 512 + tt * 128
                        tile_ap = src[col // csz][:, :, col % csz:col % csz + 128]
                        pc = pcol.tile([128, D + 1], F32, tag="pc")
                        for u in range(NU):
                            nc.tensor.matmul(
                                pc[:], tile_ap[:, u, :],
                                uea_sb[:, u // 4, u % 4, :],
                                start=(u == 0), stop=(u == NU - 1))
                        nc.vector.tensor_copy(strip[:, tt, :], pc[:])
                    nc.scalar.dma_start(
                        AGG[rbase + c * 128:rbase + (c + 1) * 128, :], strip[:])

            col_pass(acol, 512, uea2_sb, 0)
            col_pass(ocol, 512, uea3_sb, NI // 4)
            nc.gpsimd.collective_compute(
                "ReduceScatter", mybir.AluOpType.add,
                ins=[AGG.ap().opt()], outs=[RSO.ap().opt()],
                replica_groups=[list(range(M))])
            nc.gpsimd.dma_start(RSOUT[:, :], RSO[0:2 * NI // M // 4, :])

            # ---- score (attention) pass over all items, batch rows ----
            # software-pipelined: inner matmuls run 2 groups ahead of the
            # exp -> mask -> accumulate chain so no engine waits on another
            ps3 = pacc.tile([128, 3, D + 1], F32, tag="acc")
            nc.vector.memset(ps3[:], 0.0)

            ips = {}

            def inner(g):
                # items g*512 .. (g+1)*512 live on partition half g//8
                pb = 0 if g < NG // 2 else 64
                go = (g % (NG // 2)) * 512
                ip = pinner.tile([128, 512], F32, tag="ip")
                for q in range(4):
                    nc.tensor.matmul(
                        ip[:, q * 128:(q + 1) * 128],
                        iet_sb[pb:pb + 64, go + q * 128:go + (q + 1) * 128],
                        uegt_sb[pb:pb + 64, :], start=True, stop=True)
                ips[g] = ip

            LOOK = 4
            for g in range(LOOK):
                inner(g)
            for g in range(NG):
                gg, g4 = g // 4, g % 4
                ip = ips.pop(g)
                st = stp.tile([128, 512], BF16, tag="st")
                nc.scalar.activation(st[:], ip[:], ACT.Exp)
                stm = stp.tile([128, 512], BF16, tag="stm")
                nc.vector.tensor_mul(stm[:], st[:], aut_g[gg][:, g4, :])
                if g + LOOK < NG:
                    inner(g + LOOK)
                for q in range(4):
                    lq = (g == NG - 1 and q == 3)
                    iea_t = iea_g[gg][:, g4, q * (D + 1):(q + 1) * (D + 1)]
                    nc.tensor.matmul(
                        ps3[:, 0, :], stm[:, q * 128:(q + 1) * 128], iea_t,
                        start=False, stop=lq, skip_group_check=True)
                    nc.tensor.matmul(
                        ps3[:, 1, :], aut_g[gg][:, g4, q * 128:(q + 1) * 128],
                        iea_t, start=False, stop=lq, skip_group_check=True)
                    nc.tensor.matmul(
                        ps3[:, 2, :], out_g[gg][:, g4, q * 128:(q + 1) * 128],
                        iea_t, start=False, stop=lq, skip_group_check=True)

            # ---- finishing: out = func((x @ W) / (deg+EPS)) ----
            # (the deg division commutes past W, so it folds into the final
            # activation's per-partition scale)
            def recip(deg_ap, tag):
                tmp = finp.tile([128, 1], F32, tag=f"tmp{tag}")
                nc.vector.tensor_scalar_add(tmp[:], deg_ap, EPS)
                rec = finp.tile([128, 1], F32, tag=f"rec{tag}")
                nc.vector.reciprocal(rec[:], tmp[:])
                return rec

            def finish(x_bf16_sb, rec, w_ap, func, o_ap):
                pt = pcol.tile([D, 128], BF16, tag="pc")
                nc.tensor.transpose(pt[:], x_bf16_sb, id_sb[:])
                zt = finp.tile([D, 128], BF16, tag="zt")
                nc.vector.tensor_copy(zt[:], pt[:])
                ph = pcol.tile([128, D], F32, tag="pc")
                nc.tensor.matmul(ph[:], zt[:], w_ap, start=True, stop=True)
                nc.scalar.activation(o_ap, ph[:], func, scale=rec[:])

            # user-side finishing (ready as soon as the accumulators stop)
            for k, (w_ap, func, out_t, tag) in enumerate((
                    (w1_sb, ACT.Copy, H1G, "si"),
                    (w2_sb, ACT.Copy, H2G, "au"),
                    (wobs_sb, ACT.Tanh, OBSG, "ou"))):
                rec = recip(ps3[:, k, D:D + 1], tag)
                xc = finp.tile([128, D], BF16, tag=f"xc{tag}")
                nc.vector.tensor_copy(xc[:], ps3[:, k, 0:D])
                o = finp.tile([128, D], F32, tag=f"o{tag}")
                finish(xc[:], rec, w_ap, func, o[:])
                nc.scalar.dma_start(out_t[0:BSH, :], o[:])


    nc.compile()
    return nc


def _get_nc():
    if "nc" not in _CACHE:
        _CACHE["nc"] = _build()
    return _CACHE["nc"]


def _pack4(x, w):
    """[8192, w] row-major -> [2048, 4w]: row g*128+p holds items
    {512g + 128q + p : q in 0..3}, matching the score-pass tiling."""
    return np.ascontiguousarray(
        x.reshape(16, 4, 128, w).transpose(0, 2, 1, 3).reshape(2048, 4 * w))


def _prep_in_maps(users, pos_items, neg_items, adj_matrix, obs_users,
                  obs_pos_items, obs_neg_items, obs_adj_matrix, user_emb,
                  item_emb, W_1, W_2, W_obs):
    adj = np.ascontiguousarray(adj_matrix, dtype=np.float32)
    oadj = np.ascontiguousarray(obs_adj_matrix, dtype=np.float32)
    ue = np.asarray(user_emb, dtype=np.float32)
    ie = np.asarray(item_emb, dtype=np.float32)
    users = np.asarray(users).astype(np.int64)
    obs_users = np.asarray(obs_users).astype(np.int64)
    items_a = np.concatenate([np.asarray(pos_items), np.asarray(neg_items)]
                             ).astype(np.int64)
    items_o = np.concatenate([np.asarray(obs_pos_items),
                              np.asarray(obs_neg_items)]).astype(np.int64)

    iet = np.ascontiguousarray(ie.T)
    ietp = np.concatenate([iet[:, :I // 2], iet[:, I // 2:]]).astype(NPBF16)
    iea = np.concatenate([ie, np.ones((I, 1), np.float32)], axis=1)
    ieap = _pack4(iea, D + 1).astype(NPBF16)
    wpk = np.empty((D, 3 * D), np.float32)
    wpk[:, 0:D] = np.asarray(W_1, dtype=np.float32)
    wpk[:, D:2 * D] = np.asarray(W_2, dtype=np.float32)
    wpk[:, 2 * D:3 * D] = np.asarray(W_obs, dtype=np.float32)
    ident = np.eye(128, dtype=np.float32)

    in_maps = []
    for c in range(M):
        sl = slice(c * USH, (c + 1) * USH)
        bs = slice(c * BSH, (c + 1) * BSH)
        ub = users[bs]
        ob = obs_users[bs]
        uegt = np.ascontiguousarray(ue[ub].T)
        pk = np.zeros((128, 512), np.float32)
        pk[0:D, 0:BSH] = uegt
        pk[D:2 * D, 0:BSH] = uegt
        pk[:, 128:256] = ident
        pk[0:D, 256:448] = wpk
        pk[D:128, 256:448] = wpk
        uet = ue[sl].T
        uet2 = np.concatenate([uet[:, :USH // 2], uet[:, USH // 2:]])
        in_maps.append({
            "ACOL": np.ascontiguousarray(adj[sl][:, items_a]).astype(NPFP8),
            "OCOL": np.ascontiguousarray(oadj[sl][:, items_o]).astype(NPFP8),
            "UET2": np.ascontiguousarray(uet2).astype(NPBF16),
            "IETP": ietp,
            "PK": pk.astype(NPBF16),
            "AUTP": _pack4(np.ascontiguousarray(adj[ub].T), BSH).astype(NPFP8),
            "OUTP": _pack4(np.ascontiguousarray(oadj[ob].T), BSH).astype(NPFP8),
            "IEAP": ieap,
        })
    return in_maps


def _assemble(results):
    h1 = np.concatenate([np.asarray(r["H1G"], np.float32) for r in results])
    h2u = np.concatenate([np.asarray(r["H2G"], np.float32) for r in results])
    obsu = np.concatenate([np.asarray(r["OBSG"], np.float32) for r in results])
    def unpack(c):
        r = np.asarray(results[c]["RSOUT"], np.float32)
        r = r.reshape(128, 4, D + 1).transpose(1, 0, 2).reshape(512, D + 1)
        return r[:, 0:D] / (r[:, D:D + 1] + EPS)

    a_part = np.concatenate([unpack(c) for c in range(M // 2)])
    o_part = np.tanh(np.concatenate([unpack(c) for c in range(M // 2, M)]))

    h2_pos, h2_neg = a_part[:B], a_part[B:]
    obs_pos, obs_neg = o_part[:B], o_part[B:]

    def l2n(x):
        n = np.sqrt((x * x).sum(axis=1, keepdims=True))
        return x / np.maximum(n, 1e-12)

    h_user = np.tanh(np.concatenate([h1, h2u, obsu], axis=1))
    h_pos = np.tanh(np.concatenate([h2_pos, h2_pos, obs_pos], axis=1))
    h_neg = np.tanh(np.concatenate([h2_neg, h2_neg, obs_neg], axis=1))
    return l2n(h_user), l2n(h_pos), l2n(h_neg)


def kernel(users, pos_items, neg_items, adj_matrix, obs_users, obs_pos_items,
           obs_neg_items, obs_adj_matrix, iteration, user_emb, item_emb,
           W_1, W_2, W_obs):
    nc = _get_nc()
    in_maps = _prep_in_maps(users, pos_items, neg_items, adj_matrix, obs_users,
                            obs_pos_items, obs_neg_items, obs_adj_matrix,
                            user_emb, item_emb, W_1, W_2, W_obs)
    res = run_bass_kernel_spmd(nc, in_maps, core_ids=list(range(M)))
    return _assemble(res.results)
